# revision 27
# baseline (speedup 1.0000x reference)
"""Trainium2 Bass kernel: conv3x3(64->128) + ReLU + conv3x3(128->128) + ReLU + maxpool2x2.

Input  x: [32, 64, 112, 112] f32; weights w1 [128,64,3,3], w2 [128,128,3,3]; biases [128].
Output: [32, 128, 56, 56] f32.

Strategy: data-parallel over batch across 8 cores (4 images/core). Per image,
channels live on SBUF partitions and spatial positions on the free dim with a
zero-padded 114x114 layout. Each conv tap (ky,kx) is a matmul over channels at
a shifted spatial offset, accumulated in PSUM. Conv1 (K=64) packs two K=64
matmuls in the 128x128 PE array via row-group tile_position (0,0)/(64,0): the
image's top/bottom row-halves are processed concurrently from partition halves
0:64 / 64:128. Conv2 is K=128 full-array. All matmul operands are bf16 (f32
PSUM accumulate): halves DMA + SBUF traffic and enables Fast Weight Load so
LDWEIGHTS hides under the matmul stream. ReLU+bias fused in ScalarE PSUM->SBUF
copies; maxpool via two strided VectorE max ops. Image-0's input DMA is spread
across the gpsimd/sync/scalar queues so the first conv is not DMA-paced.
"""
import numpy as np
import ml_dtypes

import concourse.bass as bass
import concourse.mybir as mybir
from concourse import bacc
from concourse.tile import TileContext
from concourse.bass_utils import run_bass_kernel_spmd

N_CORES = 8
B, CIN, COUT, H, W = 32, 64, 128, 112, 112
PB = B // N_CORES            # images per core
HP = H + 2                   # padded width/height (114)
G = 128                      # zero guard columns around each padded buffer
RHALF = 58                   # padded rows held per half-region (incl. 1-row halo)
LHALF = RHALF * HP           # 6612
LXS = G + LHALF + G          # x half-region buffer length
LY1 = G + HP * HP + G       # conv1 output (padded) buffer length
NROW = 4                     # output rows per PSUM chunk
NCH = NROW * HP              # matmul free dim per chunk (456)
NR1 = (H // 2) // NROW       # conv1 chunk rounds per half (14)
NR2 = H // NROW              # conv2 chunks (28)
HO, WO = H // 2, W // 2      # pooled output dims

F32 = mybir.dt.float32
BF16 = mybir.dt.bfloat16
RELU = mybir.ActivationFunctionType.Relu

# tap offsets in padded flat coords, tap t = (ky, kx)
TAP_OFF = [(ky - 1) * HP + (kx - 1) for ky in range(3) for kx in range(3)]

_CACHE = {}

TRACE = False          # test harness may flip this for profiled runs
LAST_RESULT = None     # stashes BassKernelResults of the last run


def _build():
    nc = bacc.Bacc("TRN2", target_bir_lowering=False, debug=False,
                   num_devices=N_CORES, num_swdge_queues=4)
    x = nc.dram_tensor("x", [PB, CIN, H, W], BF16, kind="ExternalInput")
    w1t = nc.dram_tensor("w1t", [128, 9 * 128], BF16, kind="ExternalInput")
    w2t = nc.dram_tensor("w2t", [128, 9 * 128], BF16, kind="ExternalInput")
    b1 = nc.dram_tensor("b1", [128, 1], F32, kind="ExternalInput")
    b2 = nc.dram_tensor("b2", [128, 1], F32, kind="ExternalInput")
    y = nc.dram_tensor("y", [PB, COUT, HO, WO], F32, kind="ExternalOutput")

    with TileContext(nc) as tc:
        with (
            tc.tile_pool(name="const", bufs=1) as cpool,
            tc.tile_pool(name="xs", bufs=1) as xpool,
            tc.tile_pool(name="y1p", bufs=1) as y1pool,
            tc.tile_pool(name="work", bufs=4) as wpool,
            tc.tile_pool(name="oimg", bufs=2) as opool,
            tc.tile_pool(name="stage", bufs=2) as spool,
            tc.tile_pool(name="psA", bufs=2, space="PSUM") as psApool,
            tc.tile_pool(name="psB", bufs=2, space="PSUM") as psBpool,
            tc.tile_pool(name="psC", bufs=3, space="PSUM") as psCpool,
        ):
            w1sb = cpool.tile([128, 9 * 128], BF16, tag="w1")
            w2sb = cpool.tile([128, 9 * 128], BF16, tag="w2")
            b1sb = cpool.tile([128, 1], F32, tag="b1")
            b2sb = cpool.tile([128, 1], F32, tag="b2")
            # PE warmup tile: memset on gpsimd (first thing it does) so the
            # warmup matmuls can issue as soon as TensorE boots.
            warm = cpool.tile([128, NCH], BF16, tag="warm")
            nc.gpsimd.memset(warm[:, :], 0.0)
            # persistent padded buffers; borders zeroed once on gpsimd (which
            # is otherwise idle at the head) so VectorE can start re-pad
            # copies the moment image-0's staged rows land.
            xs = [xpool.tile([128, LXS], BF16, tag=f"xs{i}", name=f"xs{i}")
                  for i in range(2)]
            y1 = y1pool.tile([128, LY1], BF16, tag="y1")
            # w1 must land before conv1 round 0 (~12us): split it across the
            # heads of BOTH fast HWDGE rings. b1/b2 (tiny, needed ~14us) and
            # w2 (needed ~44us) ride the slower gpsimd SW ring.
            nc.sync.dma_start(out=w1sb[:, 0:5 * 128], in_=w1t[:, 0:5 * 128])
            nc.scalar.dma_start(out=w1sb[:, 5 * 128:], in_=w1t[:, 5 * 128:])


            def x_border_memsets(t, eng):
                tv = t[:, :]
                # guard fringe actually read: 1 elem each side (use 8)
                eng.memset(tv[:, G - 8:G], 0.0)
                eng.memset(tv[:, G + LHALF:G + LHALF + 8], 0.0)
                # pad row 0 (top halo) and row 57 (bottom halo)
                eng.memset(tv[:, G:G + HP], 0.0)
                eng.memset(tv[:, G + 57 * HP:G + 58 * HP], 0.0)
                # column borders: col 113 of row r + col 0 of row r+1, r=0..56
                cb = tv[:, G + 113:G + 113 + 57 * HP].rearrange(
                    "p (r c) -> p r c", c=HP)
                eng.memset(cb[:, :, 0:2], 0.0)

            # x staging: contiguous DRAM->SBUF loads (one 12.8KB descriptor
            # per partition, full HW DMA bandwidth), then on-chip re-pad
            # copies into the padded conv layout. partitions 0:64 hold x rows
            # 0:57 (top half), 64:128 hold x rows 55:112 (bottom half), each
            # split into a small early piece (rows :13, unblocks rounds 0-2)
            # and the remainder.
            SROW = 57                    # x rows staged per half
            RA = 13                      # rows in the early piece
            RB = 30                      # end of the second piece
            LST = SROW * W               # 6384 els per partition

            def x_stage(b, st):
                xf = x[b].rearrange("c h w -> c (h w)")
                if b == 0:
                    # image 0 needs both halves ASAP: top on the sync ring,
                    # bottom on the scalar ring (idle at the head), each in
                    # three consumption-ordered pieces
                    for r0, r1 in ((0, RA), (RA, RB), (RB, SROW)):
                        nc.sync.dma_start(out=st[0:64, r0 * W:r1 * W],
                                          in_=xf[:, r0 * W:r1 * W])
                        nc.scalar.dma_start(
                            out=st[64:128, r0 * W:r1 * W],
                            in_=xf[:, (55 + r0) * W:(55 + r1) * W])
                else:
                    # steady state: sync ring only — issuing DMAs from the
                    # scalar engine mid-stream delays its ACTIVATEs and
                    # stalls PSUM recycling
                    nc.sync.dma_start(out=st[0:64, :],
                                      in_=xf[:, 0:LST])
                    nc.sync.dma_start(out=st[64:128, :],
                                      in_=xf[:, 55 * W:112 * W])

            def x_repad(st, xv, split=False):
                # top: stage row r -> padded row r+1; bottom: row j -> row j.
                # all on VectorE: its bf16 copies hit 4x mode (~0.9-1.4us);
                # ScalarE copies measured 1x (~4.4us) so keep them off it.
                stv = st.rearrange("p (r c) -> p r c", c=W)
                if split:
                    for r0, r1 in ((0, RA), (RA, RB), (RB, SROW)):
                        nc.vector.tensor_copy(xv[0:64, r0 + 1:r1 + 1, 1:113],
                                              stv[0:64, r0:r1, :])
                        nc.vector.tensor_copy(xv[64:128, r0:r1, 1:113],
                                              stv[64:128, r0:r1, :])
                else:
                    nc.vector.tensor_copy(xv[0:64, 1:SROW + 1, 1:113],
                                          stv[0:64, :, :])
                    nc.vector.tensor_copy(xv[64:128, 0:SROW, 1:113],
                                          stv[64:128, :, :])

            x_border_memsets(xs[0], nc.gpsimd)
            xv0 = xs[0][:, G:G + LHALF].rearrange("p (r c) -> p r c", c=HP)
            stage0 = spool.tile([128, LST], BF16, tag="stage")
            x_stage(0, stage0)
            nc.gpsimd.dma_start(out=b1sb[:, :], in_=b1[:, :])
            nc.gpsimd.dma_start(out=b2sb[:, :], in_=b2[:, :])
            nc.gpsimd.dma_start(out=w2sb[:, :], in_=w2t[:, :])
            x_repad(stage0, xv0, split=True)
            x_border_memsets(xs[1], nc.gpsimd)
            y1f = y1[:, :]
            nc.vector.memset(y1f[:, G - 8:G], 0.0)
            nc.vector.memset(y1f[:, G + HP * HP:G + HP * HP + 8], 0.0)
            nc.vector.memset(y1f[:, G:G + HP], 0.0)
            nc.vector.memset(y1f[:, G + 113 * HP:G + 114 * HP], 0.0)
            y1cb = y1f[:, G + 113:G + 113 + 113 * HP].rearrange(
                "p (r c) -> p r c", c=HP)
            nc.vector.memset(y1cb[:, :, 0:2], 0.0)

            # PE warmup: zero-weight K=64 matmuls accumulating into the first
            # conv1 PSUM tile while the initial DMAs run, so the PE clock
            # gate (HAM) is at full rate when real matmuls start.
            warm_ps = psApool.tile([128, NCH], F32, tag="psA", name="warm_ps")
            N_WARM = 13
            for k in range(N_WARM):
                nc.tensor.matmul(warm_ps[:, :], warm[0:64, 0:128],
                                 warm[0:64, :], start=(k == 0), stop=False,
                                 tile_position=(0, 0))
            # image 0 / round 0 conv1 accumulates on top of the (zero) warmup
            # sums in warm_ps, so the warmup matmuls feed a live output and
            # cannot be dead-code eliminated.

            y1v = y1[:, G:G + HP * HP].rearrange("p (r c) -> p r c", c=HP)

            for b in range(PB):
                xsb = xs[b % 2]
                xv = xsb[:, G:G + LHALF].rearrange("p (r c) -> p r c", c=HP)
                if b > 0:
                    stage = spool.tile([128, LST], BF16, tag="stage")
                    x_stage(b, stage)
                    x_repad(stage, xv)

                # ---- conv1: two concurrent K=64 row-group matmul series ----
                for ri in range(NR1):
                    r = 1 + NROW * ri          # local output row base (both halves)
                    q = G + r * HP
                    warm_round = (b == 0 and ri == 0)
                    if warm_round:
                        psA = warm_ps          # continue warmup accumulation
                    else:
                        psA = psApool.tile([128, NCH], F32, tag="psA")
                    psB = psBpool.tile([128, NCH], F32, tag="psB")
                    for t in range(9):
                        off = TAP_OFF[t]
                        nc.tensor.matmul(psA[:, :],
                                         w1sb[0:64, t * 128:(t + 1) * 128],
                                         xsb[0:64, q + off:q + off + NCH],
                                         start=(t == 0 and not warm_round),
                                         stop=(t == 8),
                                         tile_position=(0, 0))
                        nc.tensor.matmul(psB[:, :],
                                         w1sb[64:128, t * 128:(t + 1) * 128],
                                         xsb[64:128, q + off:q + off + NCH],
                                         start=(t == 0), stop=(t == 8),
                                         tile_position=(64, 0))
                    pAv = psA.rearrange("p (r c) -> p r c", c=HP)
                    pBv = psB.rearrange("p (r c) -> p r c", c=HP)
                    # top half outputs: padded rows r..r+3; bottom: 56+r..56+r+3
                    nc.scalar.activation(y1v[:, r:r + NROW, 1:113],
                                         pAv[:, :, 1:113], RELU,
                                         bias=b1sb[:, 0:1])
                    nc.scalar.activation(y1v[:, 56 + r:56 + r + NROW, 1:113],
                                         pBv[:, :, 1:113], RELU,
                                         bias=b1sb[:, 0:1])

                # ---- conv2 (K=128) + fused relu + maxpool ----
                out_img = opool.tile([128, HO * WO], F32, tag="oimg")
                for ci in range(NR2):
                    r = 1 + NROW * ci          # padded output row base
                    q = G + r * HP
                    psC = psCpool.tile([128, NCH], F32, tag="psC")
                    for t in range(9):
                        off = TAP_OFF[t]
                        nc.tensor.matmul(psC[:, :],
                                         w2sb[:, t * 128:(t + 1) * 128],
                                         y1[:, q + off:q + off + NCH],
                                         start=(t == 0), stop=(t == 8))
                    y2c = wpool.tile([128, NROW * W], BF16, tag="y2c")
                    y2v = y2c.rearrange("p (r c) -> p r c", c=W)
                    pCv = psC.rearrange("p (r c) -> p r c", c=HP)
                    nc.scalar.activation(y2v[:, :, :], pCv[:, :, 1:113], RELU,
                                         bias=b2sb[:, 0:1])
                    # horizontal 2:1 max
                    hpt = wpool.tile([128, NROW * WO], BF16, tag="hp")
                    y2p = y2c.rearrange("p (r c two) -> p r c two", two=2, c=WO)
                    nc.vector.tensor_max(
                        hpt.rearrange("p (r c) -> p r c", c=WO),
                        y2p[:, :, :, 0], y2p[:, :, :, 1])
                    # vertical 2:1 max -> 2 pooled rows (f32 result)
                    hpv = hpt.rearrange("p (r two c) -> p r two c", two=2, c=WO)
                    ov = out_img[:, ci * 2 * WO:(ci * 2 + 2) * WO].rearrange(
                        "p (r c) -> p r c", c=WO)
                    nc.vector.tensor_max(ov, hpv[:, :, 0, :], hpv[:, :, 1, :])

                    yflat = y[b].rearrange("c h w -> c (h w)")
                    if b < PB - 1:
                        if ci in (13, 27):
                            # two half-image slabs per image on the gpsimd
                            # ring (free in steady state)
                            lo = 0 if ci == 13 else 14 * 2 * WO
                            hi = (ci + 1) * 2 * WO
                            nc.gpsimd.dma_start(out=yflat[:, lo:hi],
                                                in_=out_img[:, lo:hi])
                    else:
                        # final image: shrinking slabs so the tail DMA is
                        # short; split the last one across three rings
                        if ci in (13, 20, 25):
                            lo = {13: 0, 20: 14, 25: 21}[ci] * 2 * WO
                            hi = (ci + 1) * 2 * WO
                            eng = {13: nc.gpsimd, 20: nc.sync,
                                   25: nc.sync}[ci]
                            eng.dma_start(out=yflat[:, lo:hi],
                                          in_=out_img[:, lo:hi])
                        elif ci == 27:
                            # last slab split across the two HWDGE rings
                            # (gpsimd's end-of-block drain is slow)
                            lo, hi = 26 * 2 * WO, 28 * 2 * WO
                            nc.sync.dma_start(out=yflat[0:64, lo:hi],
                                              in_=out_img[0:64, lo:hi])
                            nc.scalar.dma_start(out=yflat[64:128, lo:hi],
                                                in_=out_img[64:128, lo:hi])

    nc.compile()
    return nc


def kernel(x, w1, b1, w2, b2):
    global LAST_RESULT
    x = np.ascontiguousarray(np.asarray(x, dtype=np.float32))
    w1 = np.asarray(w1, dtype=np.float32)
    w2 = np.asarray(w2, dtype=np.float32)
    b1 = np.asarray(b1, dtype=np.float32)
    b2 = np.asarray(b2, dtype=np.float32)

    if "nc" not in _CACHE:
        _CACHE["nc"] = _build()
    nc = _CACHE["nc"]

    # weight layout: w1t[ci, t*128+co] = w1[co, ci, ky, kx]; duplicated on
    # partitions 64:128 for the upper row-group. w2t likewise (full 128 rows).
    w1r = np.transpose(w1, (1, 2, 3, 0)).reshape(CIN, 9 * 128)  # ci,(ky kx co)
    w1full = np.concatenate([w1r, w1r], axis=0)                  # [128, 1152]
    w2r = np.transpose(w2, (1, 2, 3, 0)).reshape(COUT, 9 * 128)

    xb = x.astype(ml_dtypes.bfloat16)
    w1b = w1full.astype(ml_dtypes.bfloat16)
    w2b = w2r.astype(ml_dtypes.bfloat16)

    in_maps = []
    for c in range(N_CORES):
        in_maps.append({
            "x": np.ascontiguousarray(xb[c * PB:(c + 1) * PB]),
            "w1t": w1b,
            "w2t": w2b,
            "b1": b1.reshape(128, 1),
            "b2": b2.reshape(128, 1),
        })

    res = run_bass_kernel_spmd(nc, in_maps, core_ids=list(range(N_CORES)),
                               trace=TRACE)
    LAST_RESULT = res
    out = np.empty((B, COUT, HO, WO), dtype=np.float32)
    for c in range(N_CORES):
        out[c * PB:(c + 1) * PB] = res.results[c]["y"]
    return out


# revision 40
# speedup vs baseline: 1.0095x; 1.0095x over previous
"""Trainium2 Bass kernel: conv3x3(64->128) + ReLU + conv3x3(128->128) + ReLU + maxpool2x2.

Input  x: [32, 64, 112, 112] f32; weights w1 [128,64,3,3], w2 [128,128,3,3]; biases [128].
Output: [32, 128, 56, 56] f32.

Strategy: data-parallel over batch across 8 cores (4 images/core). Per image,
channels live on SBUF partitions and spatial positions on the free dim with a
zero-padded 114x114 layout. Each conv tap (ky,kx) is a matmul over channels at
a shifted spatial offset, accumulated in PSUM. Conv1 (K=64) packs two K=64
matmuls in the 128x128 PE array via row-group tile_position (0,0)/(64,0): the
image's top/bottom row-halves are processed concurrently from partition halves
0:64 / 64:128. Conv2 is K=128 full-array. All matmul operands are bf16 (f32
PSUM accumulate): halves DMA + SBUF traffic and enables Fast Weight Load so
LDWEIGHTS hides under the matmul stream (per-tap slot = 456 cols / 2.4 GHz
~ 194 ns, the MAC-rate floor). ReLU+bias fused in ScalarE PSUM->SBUF copies;
maxpool via two strided VectorE max ops.

DMA plan (all rates measured): the padded SBUF layout would force 224 B/row
packets, and every DGE ring dispatches at ~45-80 ns/packet regardless of
size, so x is instead DMA'd contiguously (12.8 KB/partition descriptors)
into an unpadded staging tile and re-padded on-chip by VectorE bf16 copies
(4x mode, ~1.7 us per half-image). Image-0's pieces are consumption-ordered
across both HWDGE rings; w1 splits across the two ring heads so conv1
round 0 is never weight-blocked; steady-state stages issue from Sync only
(a mid-stream dma_start on ScalarE delays ACTIVATEs and stalls PSUM
recycling). Zero-weight warmup matmuls bridge the boot-to-data window and
hold the PE HAM clock-gate busy so conv1 starts at the full 2.4 GHz.
"""
import numpy as np
import ml_dtypes

import concourse.bass as bass
import concourse.mybir as mybir
from concourse import bacc
from concourse.tile import TileContext
from concourse.bass_utils import run_bass_kernel_spmd

N_CORES = 8
B, CIN, COUT, H, W = 32, 64, 128, 112, 112
PB = B // N_CORES            # images per core
HP = H + 2                   # padded width/height (114)
G = 128                      # zero guard columns around each padded buffer
RHALF = 58                   # padded rows held per half-region (incl. 1-row halo)
LHALF = RHALF * HP           # 6612
LXS = G + LHALF + G          # x half-region buffer length
LY1 = G + HP * HP + G       # conv1 output (padded) buffer length
NROW = 4                     # output rows per PSUM chunk
NCH = NROW * HP              # matmul free dim per chunk (456)
NR1 = (H // 2) // NROW       # conv1 chunk rounds per half (14)
NR2 = H // NROW              # conv2 chunks (28)
HO, WO = H // 2, W // 2      # pooled output dims

F32 = mybir.dt.float32
BF16 = mybir.dt.bfloat16
RELU = mybir.ActivationFunctionType.Relu

# tap offsets in padded flat coords, tap t = (ky, kx)
TAP_OFF = [(ky - 1) * HP + (kx - 1) for ky in range(3) for kx in range(3)]

_CACHE = {}

TRACE = False          # test harness may flip this for profiled runs
LAST_RESULT = None     # stashes BassKernelResults of the last run


def _build():
    nc = bacc.Bacc("TRN2", target_bir_lowering=False, debug=False,
                   num_devices=N_CORES, num_swdge_queues=4)
    x = nc.dram_tensor("x", [PB, CIN, H, W], BF16, kind="ExternalInput")
    w1t = nc.dram_tensor("w1t", [128, 9 * 128], BF16, kind="ExternalInput")
    w2t = nc.dram_tensor("w2t", [128, 9 * 128], BF16, kind="ExternalInput")
    b1 = nc.dram_tensor("b1", [128, 1], F32, kind="ExternalInput")
    b2 = nc.dram_tensor("b2", [128, 1], F32, kind="ExternalInput")
    y = nc.dram_tensor("y", [PB, COUT, HO, WO], F32, kind="ExternalOutput")

    with TileContext(nc) as tc:
        with (
            tc.tile_pool(name="const", bufs=1) as cpool,
            tc.tile_pool(name="xs", bufs=1) as xpool,
            tc.tile_pool(name="y1p", bufs=1) as y1pool,
            tc.tile_pool(name="work", bufs=4) as wpool,
            tc.tile_pool(name="oimg", bufs=2) as opool,
            tc.tile_pool(name="stage", bufs=2) as spool,
            tc.tile_pool(name="psA", bufs=2, space="PSUM") as psApool,
            tc.tile_pool(name="psB", bufs=2, space="PSUM") as psBpool,
            tc.tile_pool(name="psC", bufs=4, space="PSUM") as psCpool,
        ):
            w1sb = cpool.tile([128, 9 * 128], BF16, tag="w1")
            w2sb = cpool.tile([128, 9 * 128], BF16, tag="w2")
            b1sb = cpool.tile([128, 1], F32, tag="b1")
            b2sb = cpool.tile([128, 1], F32, tag="b2")
            # PE warmup tile: memset on gpsimd (first thing it does) so the
            # warmup matmuls can issue as soon as TensorE boots.
            warm = cpool.tile([128, NCH], BF16, tag="warm")
            nc.gpsimd.memset(warm[:, :], 0.0)
            # persistent padded buffers; borders zeroed once on gpsimd (which
            # is otherwise idle at the head) so VectorE can start re-pad
            # copies the moment image-0's staged rows land.
            xs = [xpool.tile([128, LXS], BF16, tag=f"xs{i}", name=f"xs{i}")
                  for i in range(2)]
            y1 = y1pool.tile([128, LY1], BF16, tag="y1")



            def x_border_memsets(t, eng):
                tv = t[:, :]
                # guard fringe actually read: 1 elem each side (use 8)
                eng.memset(tv[:, G - 8:G], 0.0)
                eng.memset(tv[:, G + LHALF:G + LHALF + 8], 0.0)
                # pad row 0 (top halo) and row 57 (bottom halo)
                eng.memset(tv[:, G:G + HP], 0.0)
                eng.memset(tv[:, G + 57 * HP:G + 58 * HP], 0.0)
                # column borders: col 113 of row r + col 0 of row r+1, r=0..56
                cb = tv[:, G + 113:G + 113 + 57 * HP].rearrange(
                    "p (r c) -> p r c", c=HP)
                eng.memset(cb[:, :, 0:2], 0.0)

            # x staging: contiguous DRAM->SBUF loads (one 12.8KB descriptor
            # per partition, full HW DMA bandwidth), then on-chip re-pad
            # copies into the padded conv layout. partitions 0:64 hold x rows
            # 0:57 (top half), 64:128 hold x rows 55:112 (bottom half), each
            # split into a small early piece (rows :13, unblocks rounds 0-2)
            # and the remainder.
            SROW = 57                    # x rows staged per half
            RA = 13                      # rows in the early piece
            RB = 30                      # end of the second piece
            LST = SROW * W               # 6384 els per partition

            def x_stage(b, st):
                xf = x[b].rearrange("c h w -> c (h w)")
                if b == 0:
                    # image 0 needs both halves ASAP: top on the sync ring,
                    # bottom on the scalar ring (idle at the head), in
                    # consumption order: piece A (rounds 0-2), then w1
                    # (needed whole at round 0 but behind A's repads), then
                    # the B pieces (rounds 3+).
                    for r0, r1 in ((0, RA),):
                        nc.sync.dma_start(out=st[0:64, r0 * W:r1 * W],
                                          in_=xf[:, r0 * W:r1 * W])
                        nc.scalar.dma_start(
                            out=st[64:128, r0 * W:r1 * W],
                            in_=xf[:, (55 + r0) * W:(55 + r1) * W])
                    nc.sync.dma_start(out=w1sb[:, 0:5 * 128],
                                      in_=w1t[:, 0:5 * 128])
                    nc.scalar.dma_start(out=w1sb[:, 5 * 128:],
                                        in_=w1t[:, 5 * 128:])
                    for r0, r1 in ((RA, RB), (RB, SROW)):
                        nc.sync.dma_start(out=st[0:64, r0 * W:r1 * W],
                                          in_=xf[:, r0 * W:r1 * W])
                        nc.scalar.dma_start(
                            out=st[64:128, r0 * W:r1 * W],
                            in_=xf[:, (55 + r0) * W:(55 + r1) * W])
                else:
                    # steady state: sync ring only — issuing DMAs from the
                    # scalar engine mid-stream delays its ACTIVATEs and
                    # stalls PSUM recycling
                    nc.sync.dma_start(out=st[0:64, :],
                                      in_=xf[:, 0:LST])
                    nc.sync.dma_start(out=st[64:128, :],
                                      in_=xf[:, 55 * W:112 * W])

            def x_repad(st, xv, split=False):
                # top: stage row r -> padded row r+1; bottom: row j -> row j.
                # all on VectorE: its bf16 copies hit 4x mode (~0.9-1.4us);
                # ScalarE copies measured 1x (~4.4us) so keep them off it.
                stv = st.rearrange("p (r c) -> p r c", c=W)
                if split:
                    for r0, r1 in ((0, RA), (RA, RB), (RB, SROW)):
                        nc.vector.tensor_copy(xv[0:64, r0 + 1:r1 + 1, 1:113],
                                              stv[0:64, r0:r1, :])
                        nc.vector.tensor_copy(xv[64:128, r0:r1, 1:113],
                                              stv[64:128, r0:r1, :])
                else:
                    nc.vector.tensor_copy(xv[0:64, 1:SROW + 1, 1:113],
                                          stv[0:64, :, :])
                    nc.vector.tensor_copy(xv[64:128, 0:SROW, 1:113],
                                          stv[64:128, :, :])

            x_border_memsets(xs[0], nc.gpsimd)
            xv0 = xs[0][:, G:G + LHALF].rearrange("p (r c) -> p r c", c=HP)
            stage0 = spool.tile([128, LST], BF16, tag="stage")
            x_stage(0, stage0)
            nc.gpsimd.dma_start(out=b1sb[:, :], in_=b1[:, :])
            nc.gpsimd.dma_start(out=b2sb[:, :], in_=b2[:, :])
            nc.gpsimd.dma_start(out=w2sb[:, :], in_=w2t[:, :])
            x_repad(stage0, xv0, split=True)
            x_border_memsets(xs[1], nc.gpsimd)
            y1f = y1[:, :]
            nc.vector.memset(y1f[:, G - 8:G], 0.0)
            nc.vector.memset(y1f[:, G + HP * HP:G + HP * HP + 8], 0.0)
            nc.vector.memset(y1f[:, G:G + HP], 0.0)
            nc.vector.memset(y1f[:, G + 113 * HP:G + 114 * HP], 0.0)
            y1cb = y1f[:, G + 113:G + 113 + 113 * HP].rearrange(
                "p (r c) -> p r c", c=HP)
            nc.vector.memset(y1cb[:, :, 0:2], 0.0)

            # PE warmup: zero-weight K=64 matmuls accumulating into the first
            # conv1 PSUM tile while the initial DMAs run, so the PE clock
            # gate (HAM) is at full rate when real matmuls start.
            warm_ps = psApool.tile([128, NCH], F32, tag="psA", name="warm_ps")
            N_WARM = 13
            for k in range(N_WARM):
                nc.tensor.matmul(warm_ps[:, :], warm[0:64, 0:128],
                                 warm[0:64, :], start=(k == 0), stop=False,
                                 tile_position=(0, 0))
            # image 0 / round 0 conv1 accumulates on top of the (zero) warmup
            # sums in warm_ps, so the warmup matmuls feed a live output and
            # cannot be dead-code eliminated.

            y1v = y1[:, G:G + HP * HP].rearrange("p (r c) -> p r c", c=HP)

            for b in range(PB):
                xsb = xs[b % 2]
                xv = xsb[:, G:G + LHALF].rearrange("p (r c) -> p r c", c=HP)
                if b > 0:
                    stage = spool.tile([128, LST], BF16, tag="stage")
                    x_stage(b, stage)
                    x_repad(stage, xv)

                # ---- conv1: two concurrent K=64 row-group matmul series ----
                for ri in range(NR1):
                    r = 1 + NROW * ri          # local output row base (both halves)
                    q = G + r * HP
                    warm_round = (b == 0 and ri == 0)
                    if warm_round:
                        psA = warm_ps          # continue warmup accumulation
                    else:
                        psA = psApool.tile([128, NCH], F32, tag="psA")
                    psB = psBpool.tile([128, NCH], F32, tag="psB")
                    for t in range(9):
                        off = TAP_OFF[t]
                        nc.tensor.matmul(psA[:, :],
                                         w1sb[0:64, t * 128:(t + 1) * 128],
                                         xsb[0:64, q + off:q + off + NCH],
                                         start=(t == 0 and not warm_round),
                                         stop=(t == 8),
                                         tile_position=(0, 0))
                        nc.tensor.matmul(psB[:, :],
                                         w1sb[64:128, t * 128:(t + 1) * 128],
                                         xsb[64:128, q + off:q + off + NCH],
                                         start=(t == 0), stop=(t == 8),
                                         tile_position=(64, 0))
                    pAv = psA.rearrange("p (r c) -> p r c", c=HP)
                    pBv = psB.rearrange("p (r c) -> p r c", c=HP)
                    # top half outputs: padded rows r..r+3; bottom: 56+r..56+r+3
                    nc.scalar.activation(y1v[:, r:r + NROW, 1:113],
                                         pAv[:, :, 1:113], RELU,
                                         bias=b1sb[:, 0:1])
                    nc.scalar.activation(y1v[:, 56 + r:56 + r + NROW, 1:113],
                                         pBv[:, :, 1:113], RELU,
                                         bias=b1sb[:, 0:1])

                # ---- conv2 (K=128) + fused relu + maxpool ----
                out_img = opool.tile([128, HO * WO], F32, tag="oimg")
                for ci in range(NR2):
                    if b == PB - 1 and ci == NR2 - 1:
                        # the very last chunk runs as two 2-row pieces so
                        # most of its activate->pool->DMA chain overlaps the
                        # final matmuls instead of extending the tail
                        for h in range(2):
                            r = 1 + NROW * ci + 2 * h
                            q = G + r * HP
                            psC = psCpool.tile([128, NCH], F32, tag="psC")
                            for t in range(9):
                                off = TAP_OFF[t]
                                nc.tensor.matmul(
                                    psC[:, 0:2 * HP],
                                    w2sb[:, t * 128:(t + 1) * 128],
                                    y1[:, q + off:q + off + 2 * HP],
                                    start=(t == 0), stop=(t == 8))
                            y2c = wpool.tile([128, 2 * W], BF16, tag="y2c")
                            y2v = y2c.rearrange("p (r c) -> p r c", c=W)
                            pCv = psC[:, 0:2 * HP].rearrange(
                                "p (r c) -> p r c", c=HP)
                            nc.scalar.activation(y2v[:, :, :],
                                                 pCv[:, :, 1:113], RELU,
                                                 bias=b2sb[:, 0:1])
                            hpt = wpool.tile([128, 2 * WO], BF16, tag="hp")
                            y2p = y2c.rearrange("p (r c two) -> p r c two",
                                                two=2, c=WO)
                            nc.vector.tensor_max(
                                hpt.rearrange("p (r c) -> p r c", c=WO),
                                y2p[:, :, :, 0], y2p[:, :, :, 1])
                            hpv = hpt.rearrange("p (r two c) -> p r two c",
                                                two=2, c=WO)
                            po = (2 * ci + h) * WO
                            ov = out_img[:, po:po + WO].rearrange(
                                "p (r c) -> p r c", c=WO)
                            nc.vector.tensor_max(ov, hpv[:, :, 0, :],
                                                 hpv[:, :, 1, :])
                            yflat = y[b].rearrange("c h w -> c (h w)")
                            lo, hi = po, po + WO
                            eng = nc.sync if h == 0 else nc.scalar
                            eng.dma_start(out=yflat[:, lo:hi],
                                          in_=out_img[:, lo:hi])
                        continue
                    r = 1 + NROW * ci          # padded output row base
                    q = G + r * HP
                    psC = psCpool.tile([128, NCH], F32, tag="psC")
                    for t in range(9):
                        off = TAP_OFF[t]
                        nc.tensor.matmul(psC[:, :],
                                         w2sb[:, t * 128:(t + 1) * 128],
                                         y1[:, q + off:q + off + NCH],
                                         start=(t == 0), stop=(t == 8))
                    y2c = wpool.tile([128, NROW * W], BF16, tag="y2c")
                    y2v = y2c.rearrange("p (r c) -> p r c", c=W)
                    pCv = psC.rearrange("p (r c) -> p r c", c=HP)
                    nc.scalar.activation(y2v[:, :, :], pCv[:, :, 1:113], RELU,
                                         bias=b2sb[:, 0:1])
                    # horizontal 2:1 max
                    hpt = wpool.tile([128, NROW * WO], BF16, tag="hp")
                    y2p = y2c.rearrange("p (r c two) -> p r c two", two=2, c=WO)
                    nc.vector.tensor_max(
                        hpt.rearrange("p (r c) -> p r c", c=WO),
                        y2p[:, :, :, 0], y2p[:, :, :, 1])
                    # vertical 2:1 max -> 2 pooled rows (f32 result)
                    hpv = hpt.rearrange("p (r two c) -> p r two c", two=2, c=WO)
                    ov = out_img[:, ci * 2 * WO:(ci * 2 + 2) * WO].rearrange(
                        "p (r c) -> p r c", c=WO)
                    nc.vector.tensor_max(ov, hpv[:, :, 0, :], hpv[:, :, 1, :])

                    yflat = y[b].rearrange("c h w -> c (h w)")
                    if b < PB - 1:
                        if ci in (13, 27):
                            # two half-image slabs per image on the gpsimd
                            # ring (free in steady state)
                            lo = 0 if ci == 13 else 14 * 2 * WO
                            hi = (ci + 1) * 2 * WO
                            nc.gpsimd.dma_start(out=yflat[:, lo:hi],
                                                in_=out_img[:, lo:hi])
                    else:
                        # final image: shrinking slabs so the tail DMA is
                        # short; split the last one across three rings
                        if ci in (13, 20, 26):
                            lo = {13: 0, 20: 14, 26: 21}[ci] * 2 * WO
                            hi = (ci + 1) * 2 * WO
                            eng = {13: nc.gpsimd, 20: nc.sync,
                                   26: nc.sync}[ci]
                            eng.dma_start(out=yflat[:, lo:hi],
                                          in_=out_img[:, lo:hi])

    nc.compile()
    return nc


def kernel(x, w1, b1, w2, b2):
    global LAST_RESULT
    x = np.ascontiguousarray(np.asarray(x, dtype=np.float32))
    w1 = np.asarray(w1, dtype=np.float32)
    w2 = np.asarray(w2, dtype=np.float32)
    b1 = np.asarray(b1, dtype=np.float32)
    b2 = np.asarray(b2, dtype=np.float32)

    if "nc" not in _CACHE:
        _CACHE["nc"] = _build()
    nc = _CACHE["nc"]

    # weight layout: w1t[ci, t*128+co] = w1[co, ci, ky, kx]; duplicated on
    # partitions 64:128 for the upper row-group. w2t likewise (full 128 rows).
    w1r = np.transpose(w1, (1, 2, 3, 0)).reshape(CIN, 9 * 128)  # ci,(ky kx co)
    w1full = np.concatenate([w1r, w1r], axis=0)                  # [128, 1152]
    w2r = np.transpose(w2, (1, 2, 3, 0)).reshape(COUT, 9 * 128)

    xb = x.astype(ml_dtypes.bfloat16)
    w1b = w1full.astype(ml_dtypes.bfloat16)
    w2b = w2r.astype(ml_dtypes.bfloat16)

    in_maps = []
    for c in range(N_CORES):
        in_maps.append({
            "x": np.ascontiguousarray(xb[c * PB:(c + 1) * PB]),
            "w1t": w1b,
            "w2t": w2b,
            "b1": b1.reshape(128, 1),
            "b2": b2.reshape(128, 1),
        })

    res = run_bass_kernel_spmd(nc, in_maps, core_ids=list(range(N_CORES)),
                               trace=TRACE)
    LAST_RESULT = res
    out = np.empty((B, COUT, HO, WO), dtype=np.float32)
    for c in range(N_CORES):
        out[c * PB:(c + 1) * PB] = res.results[c]["y"]
    return out
